# revision 22
# baseline (speedup 1.0000x reference)
"""Trainium2 Bass kernel for nn_MultiHeadAttn (B=4, NQ=NK=2048, D=1024, H=8).

Sharding: 8 cores = 4 batches x 2 query-halves. Each core owns 1024 query rows
of one batch; k/v projections for that batch are computed redundantly by the
two cores sharing it (cheaper than collectives for this size).

Per-core dataflow (all activations feature-major "T layout" [feat, row]):
  qpT = Wq @ qT          (f32r ~ TF32)
  kpT = (Wk/32) @ kT     (bf16)
  vp  = v @ Wv.T         (bf16, natural [key, feat] layout)
  per head, per 512-row chunk, flash-style over 16 key tiles:
      logitsT[kk,r] = kpT_h_tile.T @ qpT_h        (bf16 matmul, PSUM f32)
      expT = Exp(logitsT + mask_bias[kk])         (ACT, per-partition bias)
      attT += vp_tile.T @ expT                     (PSUM accumulate)
      den  += ones.T @ expT                        (PSUM accumulate)
  x1T = qpT + attT / den
  out1 = LN(x1) via ones-matmul stats (sums over feature partitions)
  x2T = out1 + Relu(Wout @ out1T + bout)           (f32r matmul, ACT bias+relu)
  outT = LN(x2)  -> DRAM [feat, row]; host transposes back.
"""

from contextlib import ExitStack

import numpy as np
import ml_dtypes

import concourse.mybir as mybir
import concourse.tile as tile
from concourse import bacc
from concourse.bass_utils import run_bass_kernel_spmd

B, NQ, NK, D, H = 4, 2048, 2048, 1024, 8
DH = D // H            # 128, head dim
P = 128                # partitions
RQ = NQ // 2           # 1024 query rows per core
EPS = 1e-5

F32 = mybir.dt.float32
F32R = mybir.dt.float32r
BF16 = mybir.dt.bfloat16
BFNP = ml_dtypes.bfloat16

KT = D // P            # 8 contraction tiles over features
DT = D // P            # 8 output-feature tiles (also heads)
KKT = NK // P          # 16 key tiles
RC = RQ // 512         # 2 row chunks of 512


def build_nc():
    nc = bacc.Bacc("TRN2", target_bir_lowering=False)

    qT = nc.declare_dram_parameter("qT", [D, RQ], F32R, isOutput=False)
    kT = nc.declare_dram_parameter("kT", [D, NK], BF16, isOutput=False)
    vT = nc.declare_dram_parameter("vT", [D, NK], BF16, isOutput=False)
    wqT = nc.declare_dram_parameter("wqT", [D, D], F32R, isOutput=False)
    wkT = nc.declare_dram_parameter("wkT", [D, D], BF16, isOutput=False)
    wvT = nc.declare_dram_parameter("wvT", [D, D], BF16, isOutput=False)
    woT = nc.declare_dram_parameter("woT", [D, D], F32R, isOutput=False)
    maskb = nc.declare_dram_parameter("maskb", [P, KKT], F32, isOutput=False)
    g1 = nc.declare_dram_parameter("g1", [P, DT], F32, isOutput=False)
    b1 = nc.declare_dram_parameter("b1", [P, DT], F32, isOutput=False)
    g2 = nc.declare_dram_parameter("g2", [P, DT], F32, isOutput=False)
    b2 = nc.declare_dram_parameter("b2", [P, DT], F32, isOutput=False)
    bo = nc.declare_dram_parameter("bo", [P, DT], F32, isOutput=False)
    outT = nc.declare_dram_parameter("outT", [D, RQ], F32, isOutput=True)

    Act = mybir.ActivationFunctionType

    with tile.TileContext(nc) as tc, ExitStack() as ctx:
        consts = ctx.enter_context(tc.tile_pool(name="consts", bufs=1))
        pool_qp = ctx.enter_context(tc.tile_pool(name="pool_qp", bufs=1))

        ones_bf = consts.tile([P, P], BF16)
        nc.vector.memset(ones_bf, 1.0)
        onesn = consts.tile([P, P], BF16)
        nc.vector.memset(onesn, 1.0 / D)
        eps_sb = consts.tile([P, 1], F32)
        nc.vector.memset(eps_sb, EPS)
        maskb_sb = consts.tile([P, KKT], F32)
        nc.sync.dma_start(out=maskb_sb, in_=maskb[:, :])
        # maskones[:, kkt, :]: column kkt's 0/1 mask replicated 32 wide (bf16)
        maskones = consts.tile([P, KKT, 32], BF16)
        for t in range(KKT):
            nc.vector.tensor_scalar_mul(
                maskones[:, t, :], ones_bf[:, 0:32], maskb_sb[:, t:t + 1]
            )
        # f32r all-(1/32) for summing the 4 stacked den blocks + broadcast
        ones32r = consts.tile([P, P], F32R)
        nc.scalar.mul(ones32r, ones_bf, 1.0 / 32.0)
        g1_sb = consts.tile([P, DT], F32)
        nc.sync.dma_start(out=g1_sb, in_=g1[:, :])
        b1_sb = consts.tile([P, DT], F32)
        nc.sync.dma_start(out=b1_sb, in_=b1[:, :])
        g2_sb = consts.tile([P, DT], F32)
        nc.sync.dma_start(out=g2_sb, in_=g2[:, :])
        b2_sb = consts.tile([P, DT], F32)
        nc.sync.dma_start(out=b2_sb, in_=b2[:, :])
        bo_sb = consts.tile([P, DT], F32)
        nc.sync.dma_start(out=bo_sb, in_=bo[:, :])

        # persistent activations
        qpT_sb = pool_qp.tile([P, DT, RQ], F32)      # qp.T; becomes x1T then x2T
        xbf_sb = pool_qp.tile([P, DT, RQ], BF16)     # bf16 shadow (qp, then x1, x2)

        with (
            tc.tile_pool(name="pool_attn", bufs=1) as pool_attn,
            tc.tile_pool(name="pool_ain", bufs=1) as ain,
        ):
            kpT_sb = pool_attn.tile([P, H, NK], BF16)    # per-head [dh, key]
            vp_sb = pool_attn.tile([P, KKT, D], BF16)    # per key-tile [key, feat]
            # ------------- Phase A: projections (q, then v, then k) ----------
            # k is last so phase B (which needs kpT first) starts right after.
            # DMA slots: "qk" holds qT then kT; "vv" holds vT; "w" rotates
            # wqA -> wqB -> wv -> wk (16KB each, weight halves for q).
            with (
                tc.tile_pool(name="a_ps", bufs=3, space="PSUM") as a_ps,
                tc.tile_pool(name="vp_ps", bufs=2, space="PSUM") as vp_ps,
            ):
                wqA_sb = ain.tile([P, KT, 512], F32R, tag="w")
                qT_sb = ain.tile([P, KT, RQ], F32R, tag="qk")
                for t in range(KT):
                    nc.sync.dma_start(out=wqA_sb[:, t, :], in_=wqT[t * P:(t + 1) * P, 0:512])
                    nc.sync.dma_start(out=qT_sb[:, t, :], in_=qT[t * P:(t + 1) * P, :])
                wqB_sb = ain.tile([P, KT, 512], F32R, tag="w")
                for t in range(KT):
                    nc.sync.dma_start(out=wqB_sb[:, t, :], in_=wqT[t * P:(t + 1) * P, 512:1024])
                vT_sb = ain.tile([P, KT, NK], BF16, tag="vv")
                for t in range(KT):
                    nc.sync.dma_start(out=vT_sb[:, t, :], in_=vT[t * P:(t + 1) * P, :])

                def q_proj(w_sb, dt0):
                    for dt_ in range(dt0, dt0 + 4):
                        for c in range(RC):
                            ps = a_ps.tile([P, 512], F32, tag="aps")
                            for kt in range(KT):
                                nc.tensor.matmul(
                                    ps,
                                    w_sb[:, kt, (dt_ - dt0) * P:(dt_ - dt0 + 1) * P],
                                    qT_sb[:, kt, c * 512:(c + 1) * 512],
                                    start=(kt == 0), stop=(kt == KT - 1),
                                )
                            nc.vector.tensor_copy(qpT_sb[:, dt_, c * 512:(c + 1) * 512], ps)
                            nc.vector.tensor_copy(xbf_sb[:, dt_, c * 512:(c + 1) * 512], ps)

                q_proj(wqA_sb, 0)
                q_proj(wqB_sb, 4)

                # v projection (bf16, natural layout): vp[kk, dout] = v @ Wv.T
                # masked key rows are zeroed at drain time (mask01 per-partition).
                wvT_sb = ain.tile([P, KT, D], BF16, tag="w")
                for t in range(KT):
                    nc.sync.dma_start(out=wvT_sb[:, t, :], in_=wvT[t * P:(t + 1) * P, :])
                kT_sb = ain.tile([P, KT, NK], BF16, tag="qk")
                for t in range(KT):
                    nc.sync.dma_start(out=kT_sb[:, t, :], in_=kT[t * P:(t + 1) * P, :])
                for kkt in range(KKT):
                    for c in range(D // 512):
                        ps = vp_ps.tile([P, 512], F32, tag="vps")
                        for kt in range(KT):
                            nc.tensor.matmul(
                                ps,
                                vT_sb[:, kt, kkt * P:(kkt + 1) * P],
                                wvT_sb[:, kt, c * 512:(c + 1) * 512],
                                start=(kt == 0), stop=(kt == KT - 1),
                            )
                        nc.vector.tensor_scalar_mul(
                            vp_sb[:, kkt, c * 512:(c + 1) * 512], ps,
                            maskb_sb[:, kkt:kkt + 1],
                        )

                # k projection (bf16): kpT[dout, kk] = (Wk/32) @ k.T
                wkT_sb = ain.tile([P, KT, D], BF16, tag="w")
                for t in range(KT):
                    nc.sync.dma_start(out=wkT_sb[:, t, :], in_=wkT[t * P:(t + 1) * P, :])
                for dt_ in range(DT):
                    for c in range(NK // 512):
                        ps = a_ps.tile([P, 512], F32, tag="aps")
                        for kt in range(KT):
                            nc.tensor.matmul(
                                ps,
                                wkT_sb[:, kt, dt_ * P:(dt_ + 1) * P],
                                kT_sb[:, kt, c * 512:(c + 1) * 512],
                                start=(kt == 0), stop=(kt == KT - 1),
                            )
                        nc.vector.tensor_copy(kpT_sb[:, dt_, c * 512:(c + 1) * 512], ps)

            # ---------------- Phase B: attention ----------------
            # Mask-free exp (masked keys excluded via zeroed vp rows and
            # masked den lhsT). Emission is software-pipelined one super-group
            # (4 key tiles) ahead so the in-order PE stream never waits on ACT
            # exp latency. Denominators: 4 concurrent M=32 col-tiled matmuls
            # per burst into one PSUM tile's 32-row blocks, summed + broadcast
            # by one (1/32)-matmul, then fast-reciprocal.
            with (
                tc.tile_pool(name="att_ps", bufs=2, space="PSUM") as att_psp,
                tc.tile_pool(name="den_ps", bufs=1, space="PSUM") as den_psp,
                tc.tile_pool(name="dbc_ps", bufs=1, space="PSUM") as dbc_psp,
                tc.tile_pool(name="lg_ps", bufs=2, space="PSUM") as lg_psp,
                tc.tile_pool(name="bsb", bufs=1) as bsb,
            ):
                NSG = KKT // 4  # 4 super-groups of 4 key tiles (2 exp pairs)
                for h in range(H):
                    for c in range(RC):
                        rs = slice(c * 512, (c + 1) * 512)
                        att_ps = att_psp.tile([P, 512], F32, tag="att")
                        den_ps = den_psp.tile([P, 512], F32, tag="den")
                        exs = [None] * (2 * NSG)

                        def emit_lgexp(g):
                            lg_ps = lg_psp.tile([P, 2, 512], F32, tag="lg")
                            for j in range(2):
                                kkt = 2 * g + j
                                nc.tensor.matmul(
                                    lg_ps[:, j, :],
                                    kpT_sb[:, h, kkt * P:(kkt + 1) * P],
                                    xbf_sb[:, h, rs],
                                    start=True, stop=True,
                                )
                            ex = bsb.tile([P, 2, 512], BF16, tag="ex", bufs=4)
                            nc.scalar.activation(ex, lg_ps, Act.Exp)
                            exs[g] = ex

                        def emit_avden(sg):
                            for q in range(4):
                                kkt = 4 * sg + q
                                ex = exs[kkt // 2][:, kkt % 2, :]
                                nc.tensor.matmul(
                                    att_ps,
                                    vp_sb[:, kkt, h * DH:(h + 1) * DH],
                                    ex,
                                    start=(kkt == 0), stop=(kkt == KKT - 1),
                                )
                            for q in range(4):
                                kkt = 4 * sg + q
                                ex = exs[kkt // 2][:, kkt % 2, :]
                                nc.tensor.matmul(
                                    den_ps[32 * q:32 * (q + 1), :],
                                    maskones[:, kkt, :],
                                    ex,
                                    start=(sg == 0),
                                    stop=(sg == NSG - 1),
                                    tile_position=(0, 32 * q),
                                    skip_group_check=True,
                                )

                        emit_lgexp(0)
                        emit_lgexp(1)
                        for sg in range(1, NSG):
                            emit_lgexp(2 * sg)
                            emit_lgexp(2 * sg + 1)
                            emit_avden(sg - 1)
                        emit_avden(NSG - 1)

                        # den blocks -> f32r SBUF -> (1/32)-matmul sum+broadcast
                        dsb = bsb.tile([P, 512], F32, tag="dsb", bufs=1)
                        nc.vector.tensor_copy(dsb[:, :].bitcast(F32R), den_ps)
                        dbc = dbc_psp.tile([P, 512], F32, tag="dbc")
                        nc.tensor.matmul(
                            dbc, ones32r, dsb[:, :].bitcast(F32R),
                            start=True, stop=True,
                        )
                        rec = bsb.tile([P, 512], F32, tag="rec", bufs=1)
                        nc.vector.reciprocal_approx_fast(rec, dbc)
                        nc.vector.tensor_mul(rec, att_ps, rec)  # in-place att/den
                        # x1 = qp + att  (in place over qpT)
                        nc.vector.tensor_add(
                            qpT_sb[:, h, rs], qpT_sb[:, h, rs], rec
                        )
                        nc.vector.tensor_copy(xbf_sb[:, h, rs], qpT_sb[:, h, rs])

        # ---------------- Phase C/D: LN1, MLP, LN2 ----------------
        with (
            tc.tile_pool(name="late", bufs=1) as late,
            tc.tile_pool(name="csb", bufs=1) as csb,
            tc.tile_pool(name="c_ps", bufs=2, space="PSUM") as c_ps,
        ):
            woT_sb = late.tile([P, KT, D], F32R)
            for t in range(KT):
                nc.sync.dma_start(out=woT_sb[:, t, :], in_=woT[t * P:(t + 1) * P, :])
            x1n_sb = late.tile([P, DT, RQ], F32)

            def ln_stats_rc(src, src_bf, c):
                """stats for one row-chunk: returns (mean_ps, rsg) both
                [P,512], identical across partitions."""
                rs = slice(c * 512, (c + 1) * 512)
                mean_ps = c_ps.tile([P, 512], F32, tag="mean")
                for kt in range(KT):
                    nc.tensor.matmul(
                        mean_ps, onesn, src_bf[:, kt, rs],
                        start=(kt == 0), stop=(kt == KT - 1),
                    )
                msq_ps = c_ps.tile([P, 512], F32, tag="msq")
                for kt in range(KT):
                    sq = csb.tile([P, 512], BF16, tag="sq", bufs=3)
                    nc.scalar.square(sq, src[:, kt, rs])
                    nc.tensor.matmul(
                        msq_ps, onesn, sq,
                        start=(kt == 0), stop=(kt == KT - 1),
                    )
                musq = csb.tile([P, 512], F32, tag="musq", bufs=2)
                nc.scalar.square(musq, mean_ps)
                var = csb.tile([P, 512], F32, tag="var", bufs=2)
                nc.vector.tensor_sub(var, msq_ps, musq)
                std = csb.tile([P, 512], F32, tag="std", bufs=2)
                nc.scalar.activation(std, var, Act.Sqrt, bias=eps_sb[:, :], scale=1.0)
                rsg = csb.tile([P, 512], F32, tag="rsg", bufs=2)
                nc.vector.reciprocal_approx_fast(rsg, std)
                return mean_ps, rsg

            # LN1: x1n = LN(x1) * g1 + b1   (ACT affine also rounds to f32r)
            for c in range(RC):
                rs = slice(c * 512, (c + 1) * 512)
                mean_ps, rsg = ln_stats_rc(qpT_sb, xbf_sb, c)
                for kt in range(DT):
                    xc = csb.tile([P, 512], F32, tag="xc", bufs=3)
                    nc.vector.tensor_sub(xc, qpT_sb[:, kt, rs], mean_ps)
                    xh = csb.tile([P, 512], F32, tag="xh", bufs=3)
                    nc.vector.tensor_mul(xh, xc, rsg)
                    nc.scalar.activation(
                        x1n_sb[:, kt, rs].bitcast(F32R), xh, Act.Identity,
                        bias=b1_sb[:, kt:kt + 1], scale=g1_sb[:, kt:kt + 1],
                    )

            # MLP: x2 = x1n + relu(Wout @ x1n.T + bout)  (x2 overwrites qpT),
            # then LN2 of that row-chunk with the affine folded:
            # out = x2*(rs*g2) - (mu*(rs*g2) - b2)
            for c in range(RC):
                rs = slice(c * 512, (c + 1) * 512)
                for dt_ in range(DT):
                    z_ps = c_ps.tile([P, 512], F32, tag="z")
                    for kt in range(KT):
                        nc.tensor.matmul(
                            z_ps,
                            woT_sb[:, kt, dt_ * P:(dt_ + 1) * P],
                            x1n_sb[:, kt, rs].bitcast(F32R),
                            start=(kt == 0), stop=(kt == KT - 1),
                        )
                    rl = csb.tile([P, 512], F32, tag="rl", bufs=3)
                    nc.scalar.activation(
                        rl, z_ps, Act.Relu,
                        bias=bo_sb[:, dt_:dt_ + 1], scale=1.0,
                    )
                    nc.vector.tensor_add(qpT_sb[:, dt_, rs], x1n_sb[:, dt_, rs], rl)
                    nc.vector.tensor_copy(xbf_sb[:, dt_, rs], qpT_sb[:, dt_, rs])

                mean_ps, rsg = ln_stats_rc(qpT_sb, xbf_sb, c)
                for kt in range(DT):
                    xc = csb.tile([P, 512], F32, tag="xc", bufs=3)
                    nc.vector.tensor_sub(xc, qpT_sb[:, kt, rs], mean_ps)
                    xh = csb.tile([P, 512], F32, tag="xh", bufs=3)
                    nc.vector.tensor_mul(xh, xc, rsg)
                    ot = csb.tile([P, 512], F32, tag="ot", bufs=3)
                    nc.scalar.activation(
                        ot, xh, Act.Identity,
                        bias=b2_sb[:, kt:kt + 1], scale=g2_sb[:, kt:kt + 1],
                    )
                    nc.sync.dma_start(out=outT[kt * P:(kt + 1) * P, rs], in_=ot)

    nc.compile()
    return nc


_NC_CACHE = None


def get_nc():
    global _NC_CACHE
    if _NC_CACHE is None:
        _NC_CACHE = build_nc()
    return _NC_CACHE


def shard_inputs(q, k, v, mask, Wq, Wk, Wv, Wout, bout, g1, b1, g2, b2):
    q = np.asarray(q, dtype=np.float32)
    k = np.asarray(k, dtype=np.float32)
    v = np.asarray(v, dtype=np.float32)
    mask = np.asarray(mask)
    f32c = lambda a: np.ascontiguousarray(np.asarray(a, dtype=np.float32))
    bfc = lambda a: np.ascontiguousarray(np.asarray(a, dtype=np.float32)).astype(BFNP)
    vec = lambda a: np.ascontiguousarray(
        np.asarray(a, dtype=np.float32).reshape(DT, P).T
    )

    shared = {
        "wqT": f32c(np.asarray(Wq, np.float32).T),
        "wkT": bfc(np.asarray(Wk, np.float32).T / np.sqrt(D)),
        "wvT": bfc(np.asarray(Wv, np.float32).T),
        "woT": f32c(np.asarray(Wout, np.float32).T),
        "g1": vec(g1), "b1": vec(b1), "g2": vec(g2), "b2": vec(b2),
        "bo": vec(bout),
    }
    in_maps = []
    for c in range(8):
        b, half = divmod(c, 2)
        rows = slice(half * RQ, (half + 1) * RQ)
        mb = (~mask[b]).astype(np.float32)  # 1.0 = keep, 0.0 = masked
        in_maps.append({
            "qT": f32c(q[b, rows].T),
            "kT": bfc(k[b].T),
            "vT": bfc(v[b].T),
            "maskb": np.ascontiguousarray(mb.reshape(KKT, P).T),
            **shared,
        })
    return in_maps


def assemble_output(results):
    out = np.empty((B, NQ, D), dtype=np.float32)
    for c in range(8):
        b, half = divmod(c, 2)
        rows = slice(half * RQ, (half + 1) * RQ)
        out[b, rows, :] = results[c]["outT"].T
    return out


def kernel(**inputs):
    nc = get_nc()
    in_maps = shard_inputs(**inputs)
    res = run_bass_kernel_spmd(nc, in_maps, core_ids=list(range(8)))
    return assemble_output(res.results)
